# revision 10
# baseline (speedup 1.0000x reference)
"""Trainium2 Bass kernel for nn_BasisAffinityGAT (B=8, N=512, D=R=128, K=8).

Math (matches reference.py):
    fused = concat(desc, nve) @ W_fuse + b_fuse                 [B,N,D]
    q = fused @ W_q[k];  kk = fused @ W_k[k]                    per basis
    e_q[b,k,n] = lrelu(q).a_q[k];  e_k likewise
    logits = e_q[:,:,:,None] + e_k[:,:,None,:], symmetrized
    alpha  = softmax(logits, -1); ema update; bias_log = log(clip(ema'))

Exact algebra used (same as the earlier f32 version):
  * sym-logits[i,j] = 0.5*(s_i + s_j) with s = e_q + e_k, so the row
    softmax collapses: alpha[b,k,i,j] = softmax_j(0.5*s[b,k,:])[j],
    independent of i.
  * lrelu(x) = 0.6*x + 0.4*|x| (slope 0.2), so
    0.5*s[b,k,n] = fused[b,n,:] @ wlin[:,k]
                   + 0.2*(a_q[k] . |q_T|) + 0.2*(a_k[k] . |k_T|)
    with wlin[:,k] = 0.3*(W_q[k] @ a_q[k] + W_k[k] @ a_k[k]) host-folded.
  * b_fuse enters only through |q+bq|, |k+bk| (bq = b_fuse@W_q etc) and an
    additive constant in s that the softmax cancels.
  * bias_log content is batch-independent ([K,N,N] broadcast over B).

Sharding: core m owns basis k=m for ALL batches (K-sharded, SPMD, zero
cross-core communication).

Performance structure (memory-regime; ~358 GB/s HBM per core):
  * Outputs are written in bf16 and upcast to f32 on the host during the
    gather step (the kernel's compute is bf16 anyway; measured extra rel
    err ~1.3e-3 alpha / 2.6e-3 bias against a 2e-2 gate). This halves
    write traffic: 8.4 MiB/core instead of 16.8 MiB.
  * Read path: xb0 is its own first DMA so batch-0 compute starts ~3us
    in; b1-b3 follow on the same (sync) queue; b4-b7 + ema ride the
    gpsimd SWDGE queue so the sync queue is free for alpha writes the
    moment batch 0's softmax lands. Weights load on the scalar queue.
  * Queue plan: qSync = xb0, xr1, alpha[0..7], 2 bias quarters;
    qScalar = weights, 2 bias quarters; qGpSimd(SWDGE) = ema, xr2.
    gpsimd engine does only DMA-lib work first, then swaps once to the
    partition-broadcast ucode (lib swaps cost ~1-2us each).
  * Per-batch engine balance: Tensor 7 matmuls (q,k share one 2-bank
    PSUM tile); ACT one |qk| abs + exp+accum; DVE psum->sbuf cast
    (+b_fuse fold — the softmax cancels the resulting constant in s),
    reciprocal, p-normalize, pbar accumulate; gpsimd p-broadcast (bf16).
  * bias tail: ema is held as [128, 4N] (rows 4p+i on partition p), the
    ln(MOM*max(ema + C*pbar, EPS/MOM)) pass runs as 2 chunks of
    [128, 2N] so each DMA descriptor covers 2 consecutive HBM rows
    (2 KB descriptors), 4 DMAs split over the sync+scalar queues.
"""

import sys

import numpy as np

if "/opt/trn_rl_repo" not in sys.path:
    sys.path.insert(0, "/opt/trn_rl_repo")

from contextlib import ExitStack

import ml_dtypes

import concourse.bass as bass
import concourse.tile as tile
from concourse import bacc, mybir
from concourse.bass_utils import run_bass_kernel_spmd

B, N, D, K = 8, 512, 128, 8
R = D
MOM = 0.99
EPS = 1e-6
N_CORES = 8
F32 = mybir.dt.float32
BF16 = mybir.dt.bfloat16
AF = mybir.ActivationFunctionType
ALU = mybir.AluOpType
NPBF = ml_dtypes.bfloat16

WCOLS = 2 * R + 3  # wq | wk | 0.2*aq | 0.2*ak | wlin


def build():
    """Build the SPMD per-core Bass program (identical on all 8 cores)."""
    nc = bacc.Bacc("TRN2", target_bir_lowering=False, debug=False,
                   num_devices=N_CORES)

    # ---- per-core external tensors -------------------------------------
    # xT[d, b*2N + h*N + n]: h=0 desc[b].T, h=1 nve[b].T (bf16, shared)
    xT = nc.dram_tensor("xT", [D, B * 2 * N], BF16, kind="ExternalInput")
    wfuse = nc.dram_tensor("wfuse", [D, 2 * D], BF16, kind="ExternalInput")
    wpack = nc.dram_tensor("wpack", [D, WCOLS], BF16, kind="ExternalInput")
    bfuse = nc.dram_tensor("bfuse", [D, 1], F32, kind="ExternalInput")
    ema = nc.dram_tensor("ema", [N, N], BF16, kind="ExternalInput")  # [m]
    alpha = nc.dram_tensor("alpha", [B, N, N], BF16, kind="ExternalOutput")
    biaso = nc.dram_tensor("bias", [B, N, N], BF16, kind="ExternalOutput")

    NB0 = 1   # batches in the first x DMA
    NB1 = 3   # batches in the second x DMA (sync queue)

    with ExitStack() as ctx:
        tc = ctx.enter_context(tile.TileContext(nc))
        const = ctx.enter_context(tc.tile_pool(name="const", bufs=1))
        work = ctx.enter_context(tc.tile_pool(name="work", bufs=2))
        absp = ctx.enter_context(tc.tile_pool(name="absp", bufs=4))
        psum = ctx.enter_context(tc.tile_pool(name="psum", bufs=1, space="PSUM"))

        wfuse_sb = const.tile([D, 2 * D], BF16)
        wpack_sb = const.tile([D, WCOLS], BF16)
        bfuse_sb = const.tile([D, 1], F32)
        ema_sb = const.tile([128, 4 * N], BF16)
        x0_sb = const.tile([D, NB0 * 2 * N], BF16)
        x1_sb = const.tile([D, NB1 * 2 * N], BF16)
        x2_sb = const.tile([D, (B - NB0 - NB1) * 2 * N], BF16)
        pbs1 = const.tile([1, N], F32)
        pbs2 = const.tile([1, 2 * N], F32)
        warm_sb = const.tile([1, 16], BF16)
        warm2_sb = const.tile([1, 16], F32)
        warm3_sb = const.tile([8, 16], F32)

        # ---- input DMAs ------------------------------------------------
        # gpsimd: tiny SWDGE warm-up, then the two non-urgent reads, then
        # a partition-broadcast warm-up (one ucode lib swap, early).
        nc.gpsimd.dma_start(warm_sb[:], wfuse[0:1, 0:16])
        nc.gpsimd.dma_start(
            ema_sb[:].rearrange("p (i n) -> p i n", i=4),
            ema.ap().rearrange("(p i) n -> p i n", p=128))
        nc.gpsimd.dma_start(x2_sb[:], xT[:, (NB0 + NB1) * 2 * N:])
        nc.vector.memset(warm2_sb[:], 0.0)
        nc.gpsimd.partition_broadcast(warm3_sb[:], warm2_sb[:], 8)
        # sync: batch-0 x first (gates the first matmul), then b1-b3.
        nc.sync.dma_start(x0_sb[:], xT[:, 0:NB0 * 2 * N])
        nc.sync.dma_start(x1_sb[:], xT[:, NB0 * 2 * N:(NB0 + NB1) * 2 * N])
        # scalar: weights
        nc.scalar.dma_start(wfuse_sb[:], wfuse[:])
        nc.scalar.dma_start(wpack_sb[:], wpack[:])
        nc.scalar.dma_start(bfuse_sb[:], bfuse[:])

        wq_ap = wpack_sb[:, 0:R]
        wk_ap = wpack_sb[:, R:2 * R]
        aq_ap = wpack_sb[:, 2 * R:2 * R + 1]
        ak_ap = wpack_sb[:, 2 * R + 1:2 * R + 2]
        wlin_ap = wpack_sb[:, 2 * R + 2:2 * R + 3]
        bf_ap = bfuse_sb[:, 0:1]

        def x_ap(b):
            if b < NB0:
                return x0_sb[:, b * 2 * N:(b + 1) * 2 * N]
            if b < NB0 + NB1:
                bb = b - NB0
                return x1_sb[:, bb * 2 * N:(bb + 1) * 2 * N]
            bb = b - NB0 - NB1
            return x2_sb[:, bb * 2 * N:(bb + 1) * 2 * N]

        for b in range(B):
            xb = x_ap(b)
            psum_f = psum.tile([D, N], F32, tag="mm", bufs=2)
            nc.tensor.matmul(psum_f[:], wfuse_sb[:, 0:D], xb[:, 0:N],
                             start=True, stop=False)
            nc.tensor.matmul(psum_f[:], wfuse_sb[:, D:2 * D],
                             xb[:, N:2 * N], start=False, stop=True)
            # fused = psum + b_fuse (the wlin.b_fuse constant in s cancels
            # in the softmax; the q/k projections get their bias through
            # fused itself).
            fused_sb = absp.tile([D, N], BF16, tag="fused", bufs=3)
            nc.vector.tensor_scalar_add(fused_sb[:], psum_f[:], bf_ap)
            psum_s = psum.tile([1, N], F32, tag="ps", bufs=2)
            nc.tensor.matmul(psum_s[:], wlin_ap, fused_sb[:],
                             start=True, stop=False)
            # q and k land side by side in one 2-bank PSUM tile so a
            # single ACT Abs covers both.
            psum_qk = psum.tile([D, 2 * N], F32, tag="qk", bufs=2)
            nc.tensor.matmul(psum_qk[:, 0:N], wq_ap, fused_sb[:],
                             start=True, stop=True)
            nc.tensor.matmul(psum_qk[:, N:2 * N], wk_ap, fused_sb[:],
                             start=True, stop=True)
            absqk = absp.tile([D, 2 * N], BF16, tag="abs", bufs=3)
            nc.scalar.activation(absqk[:], psum_qk[:], AF.Abs)
            nc.tensor.matmul(psum_s[:], aq_ap, absqk[:, 0:N],
                             start=False, stop=False)
            nc.tensor.matmul(psum_s[:], ak_ap, absqk[:, N:2 * N],
                             start=False, stop=True)

            # ---- softmax over free dim (no max-shift: |s| is O(3)) -----
            expv = work.tile([1, N], F32, tag="ex", bufs=4)
            sume = work.tile([1, 1], F32, tag="se", bufs=4)
            nc.scalar.activation(expv[:], psum_s[:], AF.Exp,
                                 scale=1.0, accum_out=sume[:])
            rsum = work.tile([1, 1], F32, tag="rs", bufs=4)
            nc.vector.reciprocal(rsum[:], sume[:])

            # p_b (bf16) on one partition, replicated to 128 by gpsimd.
            pnorm = work.tile([1, N], BF16, tag="pn", bufs=4)
            nc.vector.tensor_scalar_mul(pnorm[:], expv[:], rsum[:])
            # pbar partial sum on [1,N] f32 (DVE)
            if b == 0:
                nc.vector.tensor_scalar_mul(pbs1[:], expv[:], rsum[:])
            else:
                nc.vector.scalar_tensor_tensor(
                    pbs1[:], expv[:], rsum[:], pbs1[:],
                    op0=ALU.mult, op1=ALU.add)
            rep_t = absp.tile([128, N], BF16, tag="repsb", bufs=4)
            nc.gpsimd.partition_broadcast(rep_t[:], pnorm[:], 128)
            src = rep_t[:].rearrange(
                "p (o n) -> p o n", o=1).broadcast_to([128, 4, N])
            dst = alpha[b].rearrange("(p i) j -> p i j", p=128)
            # sync engine is idle after its two read issues; a dma_start
            # stalls the issuing engine until the source tile is ready,
            # which is free there.
            nc.sync.dma_start(dst, src)

        # ---- bias_log tail ---------------------------------------------
        # pbs2 = [pbar, pbar]; broadcast once to 128 partitions; then two
        # [128, 2N] chunks (ema rows 4p+2h, 4p+2h+1 -> 2KB descriptors).
        nc.vector.tensor_copy(pbs2[:, 0:N], pbs1[:])
        nc.vector.tensor_copy(pbs2[:, N:2 * N], pbs1[:])
        pb_sb = absp.tile([128, 2 * N], F32, tag="pbb", bufs=1)
        nc.gpsimd.partition_broadcast(pb_sb[:], pbs2[:], 128)
        dst_all = biaso.ap().rearrange("b (p g x) j -> g p b (x j)", g=2, x=2)
        bias_q = [[nc.scalar, nc.sync], [nc.scalar, nc.sync]]
        for h in range(2):
            sl = slice(2 * h * N, 2 * (h + 1) * N)
            u = work.tile([128, 2 * N], F32, tag="u", bufs=2)
            nc.vector.scalar_tensor_tensor(
                u[:], pb_sb[:], 0.01 / B / MOM,
                ema_sb[:, sl], op0=ALU.mult, op1=ALU.add)
            v = work.tile([128, 2 * N], F32, tag="v", bufs=2)
            nc.vector.tensor_scalar_max(v[:], u[:], EPS / MOM)
            bias_t = work.tile([128, 2 * N], BF16, tag="biassb", bufs=2)
            nc.scalar.activation(bias_t[:], v[:], AF.Ln, scale=MOM)
            src = bias_t[:].rearrange(
                "p (o m) -> p o m", o=1).broadcast_to([128, B, 2 * N])
            for half in range(2):
                bsl = slice(half * 4, (half + 1) * 4)
                bias_q[h][half].dma_start(dst_all[h][:, bsl],
                                          src[:, bsl])

    nc.compile()
    return nc


_NC_CACHE = None


def _get_nc():
    global _NC_CACHE
    if _NC_CACHE is None:
        _NC_CACHE = build()
    return _NC_CACHE


def make_in_maps(desc_embeddings, name_value_embeddings, W_fuse, b_fuse,
                 W_q, W_k, a, alpha_ema):
    """Host-side sharding / weight prep -> per-core input dicts."""
    desc = np.asarray(desc_embeddings, np.float32)
    nve = np.asarray(name_value_embeddings, np.float32)
    W_fuse = np.asarray(W_fuse, np.float32)
    b_fuse = np.asarray(b_fuse, np.float32)
    W_q = np.asarray(W_q, np.float32)
    W_k = np.asarray(W_k, np.float32)
    a = np.asarray(a, np.float32)
    alpha_ema = np.asarray(alpha_ema, np.float32)

    a_q = a[:, :R, 0]                      # [K,R]
    a_k = a[:, R:, 0]                      # [K,R]
    wlin = 0.3 * (np.einsum("kdr,kr->kd", W_q, a_q)
                  + np.einsum("kdr,kr->kd", W_k, a_k))  # [K,D]

    # xT[d, b*2N + h*N + n]: per-partition-contiguous, batch-major
    xT = np.empty((D, B, 2, N), np.float32)
    xT[:, :, 0, :] = desc.transpose(2, 0, 1)
    xT[:, :, 1, :] = nve.transpose(2, 0, 1)
    xT = np.ascontiguousarray(xT.reshape(D, B * 2 * N)).astype(NPBF)
    # wfuse_sb[c, h*D+d] = W_fuse[h*D+c, d]
    wfuse_t = np.ascontiguousarray(
        W_fuse.reshape(2, D, D).transpose(1, 0, 2).reshape(D, 2 * D)
    ).astype(NPBF)

    shared = dict(xT=xT, wfuse=wfuse_t,
                  bfuse=np.ascontiguousarray(b_fuse[:, None], np.float32))
    in_maps = []
    for m in range(N_CORES):
        wp = np.concatenate(
            [W_q[m], W_k[m], 0.2 * a_q[m][:, None], 0.2 * a_k[m][:, None],
             wlin[m][:, None]], axis=1)
        in_maps.append(dict(
            shared,
            wpack=np.ascontiguousarray(wp).astype(NPBF),
            ema=np.ascontiguousarray(alpha_ema[m]).astype(NPBF)))
    return in_maps


def gather(results):
    alpha_full = np.stack(
        [np.asarray(r["alpha"]).astype(np.float32) for r in results], axis=1)
    bias_full = np.stack(
        [np.asarray(r["bias"]).astype(np.float32) for r in results], axis=1)
    return bias_full, alpha_full


def kernel(**inputs):
    nc = _get_nc()
    in_maps = make_in_maps(**inputs)
    res = run_bass_kernel_spmd(nc, in_maps, list(range(N_CORES)))
    return gather(res.results)


# revision 17
# speedup vs baseline: 1.1616x; 1.1616x over previous
"""Trainium2 Bass kernel for nn_BasisAffinityGAT (B=8, N=512, D=R=128, K=8).

Math (matches reference.py):
    fused = concat(desc, nve) @ W_fuse + b_fuse                 [B,N,D]
    q = fused @ W_q[k];  kk = fused @ W_k[k]                    per basis
    e_q[b,k,n] = lrelu(q).a_q[k];  e_k likewise
    logits = e_q[:,:,:,None] + e_k[:,:,None,:], symmetrized
    alpha  = softmax(logits, -1); ema update; bias_log = log(clip(ema'))

Exact algebra used (same as the earlier f32 version):
  * sym-logits[i,j] = 0.5*(s_i + s_j) with s = e_q + e_k, so the row
    softmax collapses: alpha[b,k,i,j] = softmax_j(0.5*s[b,k,:])[j],
    independent of i.
  * lrelu(x) = 0.6*x + 0.4*|x| (slope 0.2), so
    0.5*s[b,k,n] = fused[b,n,:] @ wlin[:,k]
                   + 0.2*(a_q[k] . |q_T|) + 0.2*(a_k[k] . |k_T|)
    with wlin[:,k] = 0.3*(W_q[k] @ a_q[k] + W_k[k] @ a_k[k]) host-folded.
  * b_fuse enters only through |q+bq|, |k+bk| (bq = b_fuse@W_q etc) and an
    additive constant in s that the softmax cancels.
  * bias_log content is batch-independent ([K,N,N] broadcast over B).

Sharding: core m owns basis k=m for ALL batches (K-sharded, SPMD, zero
cross-core communication).

Performance structure (memory-regime; ~358 GB/s HBM per core):
  * Outputs are written in bf16 and upcast to f32 on the host during the
    gather step (the kernel's compute is bf16 anyway; measured extra rel
    err ~1.3e-3 alpha / 2.6e-3 bias against a 2e-2 gate). This halves
    write traffic: 8.4 MiB/core instead of 16.8 MiB.
  * Read path: xb0 is its own first DMA so batch-0 compute starts ~3us
    in; b1-b3 follow on the same (sync) queue; b4-b7 + ema ride the
    gpsimd SWDGE queue so the sync queue is free for alpha writes the
    moment batch 0's softmax lands. Weights load on the scalar queue.
  * Queue plan: qSync = xb0, xr1, alpha[0..7], 2 bias quarters;
    qScalar = weights, 2 bias quarters; qGpSimd(SWDGE) = ema, xr2.
    gpsimd engine does only DMA-lib work first, then swaps once to the
    partition-broadcast ucode (lib swaps cost ~1-2us each).
  * Per-batch engine balance: Tensor 7 matmuls (q,k share one 2-bank
    PSUM tile); ACT one |qk| abs + exp+accum; DVE psum->sbuf cast
    (+b_fuse fold — the softmax cancels the resulting constant in s),
    reciprocal, p-normalize, pbar accumulate; gpsimd p-broadcast (bf16).
  * bias tail: ema is held as [128, 4N] (rows 4p+i on partition p), the
    ln(MOM*max(ema + C*pbar, EPS/MOM)) pass runs as 2 chunks of
    [128, 2N] so each DMA descriptor covers 2 consecutive HBM rows
    (2 KB descriptors), 4 DMAs split over the sync+scalar queues.
"""

import sys

import numpy as np

if "/opt/trn_rl_repo" not in sys.path:
    sys.path.insert(0, "/opt/trn_rl_repo")

from contextlib import ExitStack

import ml_dtypes

import concourse.bass as bass
import concourse.tile as tile
from concourse import bacc, mybir
from concourse.bass_utils import run_bass_kernel_spmd

B, N, D, K = 8, 512, 128, 8
R = D
MOM = 0.99
EPS = 1e-6
N_CORES = 8
F32 = mybir.dt.float32
BF16 = mybir.dt.bfloat16
AF = mybir.ActivationFunctionType
ALU = mybir.AluOpType
NPBF = ml_dtypes.bfloat16

WCOLS = 2 * R + 3  # wq | wk | 0.2*aq | 0.2*ak | wlin


def build():
    """Build the SPMD per-core Bass program (identical on all 8 cores)."""
    nc = bacc.Bacc("TRN2", target_bir_lowering=False, debug=False,
                   num_devices=N_CORES)

    # ---- per-core external tensors -------------------------------------
    # xT[d, b*2N + h*N + n]: h=0 desc[b].T, h=1 nve[b].T (bf16, shared)
    xT = nc.dram_tensor("xT", [D, B * 2 * N], BF16, kind="ExternalInput")
    wfuse = nc.dram_tensor("wfuse", [D, 2 * D], BF16, kind="ExternalInput")
    wpack = nc.dram_tensor("wpack", [D, WCOLS], BF16, kind="ExternalInput")
    # b_fuse as a single-partition row: one 512B DMA descriptor. A [D,1]
    # column load is 128 4-byte descriptors and takes ~8.5us to complete.
    bfuse = nc.dram_tensor("bfuse", [1, D], F32, kind="ExternalInput")
    ema = nc.dram_tensor("ema", [N, N], BF16, kind="ExternalInput")  # [m]
    alpha = nc.dram_tensor("alpha", [B, N, N], BF16, kind="ExternalOutput")
    biaso = nc.dram_tensor("bias", [B, N, N], BF16, kind="ExternalOutput")

    NB0 = 1   # batches in the first x DMA
    NB1 = 3   # batches in the second x DMA (sync queue)

    with ExitStack() as ctx:
        tc = ctx.enter_context(tile.TileContext(nc))
        const = ctx.enter_context(tc.tile_pool(name="const", bufs=1))
        work = ctx.enter_context(tc.tile_pool(name="work", bufs=2))
        absp = ctx.enter_context(tc.tile_pool(name="absp", bufs=4))
        psum = ctx.enter_context(tc.tile_pool(name="psum", bufs=1, space="PSUM"))

        wfuse_sb = const.tile([D, 2 * D], BF16)
        wpack_sb = const.tile([D, WCOLS], BF16)
        bfr_sb = const.tile([1, D], F32)
        bfuse_sb = const.tile([D, 1], F32)
        ones1_sb = const.tile([1, 1], F32)
        ema_sb = const.tile([128, 4 * N], BF16)
        x0_sb = const.tile([D, NB0 * 2 * N], BF16)
        x1_sb = const.tile([D, NB1 * 2 * N], BF16)
        x2_sb = const.tile([D, (B - NB0 - NB1) * 2 * N], BF16)
        pbs1 = const.tile([1, N], F32)
        warm_sb = const.tile([1, 16], BF16)
        warm2_sb = const.tile([1, 16], F32)
        warm3_sb = const.tile([8, 16], F32)

        # ---- input DMAs ------------------------------------------------
        # gpsimd: tiny SWDGE warm-up, then the two non-urgent reads, then
        # a partition-broadcast warm-up (one ucode lib swap, early).
        nc.gpsimd.dma_start(warm_sb[:], wfuse[0:1, 0:16])
        nc.gpsimd.dma_start(
            ema_sb[:].rearrange("p (i n) -> p i n", i=4),
            ema.ap().rearrange("(p i) n -> p i n", p=128))
        nc.gpsimd.dma_start(x2_sb[:], xT[:, (NB0 + NB1) * 2 * N:])
        nc.vector.memset(warm2_sb[:], 0.0)
        nc.gpsimd.partition_broadcast(warm3_sb[:], warm2_sb[:], 8)
        # sync: batch-0 x first (gates the first matmul), then b1-b3.
        nc.sync.dma_start(x0_sb[:], xT[:, 0:NB0 * 2 * N])
        nc.sync.dma_start(x1_sb[:], xT[:, NB0 * 2 * N:(NB0 + NB1) * 2 * N])
        # scalar: weights
        nc.scalar.dma_start(wfuse_sb[:], wfuse[:])
        nc.scalar.dma_start(bfr_sb[:], bfuse[:])
        nc.scalar.dma_start(wpack_sb[:], wpack[:])
        nc.vector.memset(ones1_sb[:], 1.0)
        # transpose b_fuse row -> column on the PE: out[d,0] = bfr[0,d]*1
        # (borrows an mm psum buffer; batch 0 rotates to the next one)
        psum_bf = psum.tile([D, N], F32, tag="mm", bufs=2)
        nc.tensor.matmul(psum_bf[:, 0:1], bfr_sb[:], ones1_sb[:],
                         start=True, stop=True)
        nc.vector.tensor_copy(bfuse_sb[:], psum_bf[:, 0:1])

        wq_ap = wpack_sb[:, 0:R]
        wk_ap = wpack_sb[:, R:2 * R]
        aq_ap = wpack_sb[:, 2 * R:2 * R + 1]
        ak_ap = wpack_sb[:, 2 * R + 1:2 * R + 2]
        wlin_ap = wpack_sb[:, 2 * R + 2:2 * R + 3]
        bf_ap = bfuse_sb[:, 0:1]

        def x_ap(b):
            if b < NB0:
                return x0_sb[:, b * 2 * N:(b + 1) * 2 * N]
            if b < NB0 + NB1:
                bb = b - NB0
                return x1_sb[:, bb * 2 * N:(bb + 1) * 2 * N]
            bb = b - NB0 - NB1
            return x2_sb[:, bb * 2 * N:(bb + 1) * 2 * N]

        for b in range(B):
            xb = x_ap(b)
            psum_f = psum.tile([D, N], F32, tag="mm", bufs=2)
            nc.tensor.matmul(psum_f[:], wfuse_sb[:, 0:D], xb[:, 0:N],
                             start=True, stop=False)
            nc.tensor.matmul(psum_f[:], wfuse_sb[:, D:2 * D],
                             xb[:, N:2 * N], start=False, stop=True)
            # fused = psum + b_fuse (the wlin.b_fuse constant in s cancels
            # in the softmax; the q/k projections get their bias through
            # fused itself).
            fused_sb = absp.tile([D, N], BF16, tag="fused", bufs=3)
            nc.vector.tensor_scalar_add(fused_sb[:], psum_f[:], bf_ap)
            psum_s = psum.tile([1, N], F32, tag="ps", bufs=2)
            nc.tensor.matmul(psum_s[:], wlin_ap, fused_sb[:],
                             start=True, stop=False)
            # q and k land side by side in one 2-bank PSUM tile so a
            # single ACT Abs covers both.
            psum_qk = psum.tile([D, 2 * N], F32, tag="qk", bufs=2)
            nc.tensor.matmul(psum_qk[:, 0:N], wq_ap, fused_sb[:],
                             start=True, stop=True)
            nc.tensor.matmul(psum_qk[:, N:2 * N], wk_ap, fused_sb[:],
                             start=True, stop=True)
            absqk = absp.tile([D, 2 * N], BF16, tag="abs", bufs=3)
            nc.scalar.activation(absqk[:], psum_qk[:], AF.Abs)
            nc.tensor.matmul(psum_s[:], aq_ap, absqk[:, 0:N],
                             start=False, stop=False)
            nc.tensor.matmul(psum_s[:], ak_ap, absqk[:, N:2 * N],
                             start=False, stop=True)

            # ---- softmax over free dim (no max-shift: |s| is O(3)) -----
            expv = work.tile([1, N], F32, tag="ex", bufs=4)
            sume = work.tile([1, 1], F32, tag="se", bufs=4)
            nc.scalar.activation(expv[:], psum_s[:], AF.Exp,
                                 scale=1.0, accum_out=sume[:])
            rsum = work.tile([1, 1], F32, tag="rs", bufs=4)
            nc.vector.reciprocal(rsum[:], sume[:])

            # p_b (bf16) on one partition, replicated to 128 by gpsimd.
            pnorm = work.tile([1, N], BF16, tag="pn", bufs=4)
            nc.vector.tensor_scalar_mul(pnorm[:], expv[:], rsum[:])
            # pbar partial sum on [1,N] f32 (DVE)
            if b == 0:
                nc.vector.tensor_scalar_mul(pbs1[:], expv[:], rsum[:])
            else:
                nc.vector.scalar_tensor_tensor(
                    pbs1[:], expv[:], rsum[:], pbs1[:],
                    op0=ALU.mult, op1=ALU.add)
            rep_t = absp.tile([128, N], BF16, tag="repsb", bufs=4)
            nc.gpsimd.partition_broadcast(rep_t[:], pnorm[:], 128)
            src = rep_t[:].rearrange(
                "p (o n) -> p o n", o=1).broadcast_to([128, 4, N])
            dst = alpha[b].rearrange("(p i) j -> p i j", p=128)
            # sync engine is idle after its two read issues; a dma_start
            # stalls the issuing engine until the source tile is ready,
            # which is free there.
            nc.sync.dma_start(dst, src)

        # ---- bias_log tail ---------------------------------------------
        # pb_sb[p,n] = pbar[n] on every partition (one gpsimd bcast);
        # bias = ln(MOM * max(ema + C*pbar, EPS/MOM)), 4 pipelined chunks.
        pb_sb = absp.tile([128, N], F32, tag="pbb", bufs=1)
        nc.gpsimd.partition_broadcast(pb_sb[:], pbs1[:], 128)
        dst_all = biaso.ap().rearrange("b (p i) j -> i p b j", i=4)
        bias_q = [nc.scalar, nc.sync, nc.scalar, nc.sync]
        for i in range(4):
            sl = slice(i * N, (i + 1) * N)
            u = work.tile([128, N], F32, tag="u", bufs=4)
            nc.vector.scalar_tensor_tensor(
                u[:], pb_sb[:], 0.01 / B / MOM,
                ema_sb[:, sl], op0=ALU.mult, op1=ALU.add)
            v = work.tile([128, N], F32, tag="v", bufs=4)
            nc.vector.tensor_scalar_max(v[:], u[:], EPS / MOM)
            bias_t = work.tile([128, N], BF16, tag="biassb", bufs=4)
            nc.scalar.activation(bias_t[:], v[:], AF.Ln, scale=MOM)
            src = bias_t[:].rearrange(
                "p (o j) -> p o j", o=1).broadcast_to([128, B, N])
            bias_q[i].dma_start(dst_all[i], src)

    nc.compile()
    return nc


_NC_CACHE = None


def _get_nc():
    global _NC_CACHE
    if _NC_CACHE is None:
        _NC_CACHE = build()
    return _NC_CACHE


def make_in_maps(desc_embeddings, name_value_embeddings, W_fuse, b_fuse,
                 W_q, W_k, a, alpha_ema):
    """Host-side sharding / weight prep -> per-core input dicts."""
    desc = np.asarray(desc_embeddings, np.float32)
    nve = np.asarray(name_value_embeddings, np.float32)
    W_fuse = np.asarray(W_fuse, np.float32)
    b_fuse = np.asarray(b_fuse, np.float32)
    W_q = np.asarray(W_q, np.float32)
    W_k = np.asarray(W_k, np.float32)
    a = np.asarray(a, np.float32)
    alpha_ema = np.asarray(alpha_ema, np.float32)

    a_q = a[:, :R, 0]                      # [K,R]
    a_k = a[:, R:, 0]                      # [K,R]
    wlin = 0.3 * (np.einsum("kdr,kr->kd", W_q, a_q)
                  + np.einsum("kdr,kr->kd", W_k, a_k))  # [K,D]

    # xT[d, b*2N + h*N + n]: per-partition-contiguous, batch-major
    xT = np.empty((D, B, 2, N), np.float32)
    xT[:, :, 0, :] = desc.transpose(2, 0, 1)
    xT[:, :, 1, :] = nve.transpose(2, 0, 1)
    xT = np.ascontiguousarray(xT.reshape(D, B * 2 * N)).astype(NPBF)
    # wfuse_sb[c, h*D+d] = W_fuse[h*D+c, d]
    wfuse_t = np.ascontiguousarray(
        W_fuse.reshape(2, D, D).transpose(1, 0, 2).reshape(D, 2 * D)
    ).astype(NPBF)

    shared = dict(xT=xT, wfuse=wfuse_t,
                  bfuse=np.ascontiguousarray(b_fuse[None, :], np.float32))
    in_maps = []
    for m in range(N_CORES):
        wp = np.concatenate(
            [W_q[m], W_k[m], 0.2 * a_q[m][:, None], 0.2 * a_k[m][:, None],
             wlin[m][:, None]], axis=1)
        in_maps.append(dict(
            shared,
            wpack=np.ascontiguousarray(wp).astype(NPBF),
            ema=np.ascontiguousarray(alpha_ema[m]).astype(NPBF)))
    return in_maps


def gather(results):
    alpha_full = np.stack(
        [np.asarray(r["alpha"]).astype(np.float32) for r in results], axis=1)
    bias_full = np.stack(
        [np.asarray(r["bias"]).astype(np.float32) for r in results], axis=1)
    return bias_full, alpha_full


def kernel(**inputs):
    nc = _get_nc()
    in_maps = make_in_maps(**inputs)
    res = run_bass_kernel_spmd(nc, in_maps, list(range(N_CORES)))
    return gather(res.results)


# revision 25
# speedup vs baseline: 1.2024x; 1.0352x over previous
"""Trainium2 Bass kernel for nn_BasisAffinityGAT (B=8, N=512, D=R=128, K=8).

Math (matches reference.py):
    fused = concat(desc, nve) @ W_fuse + b_fuse                 [B,N,D]
    q = fused @ W_q[k];  kk = fused @ W_k[k]                    per basis
    e_q[b,k,n] = lrelu(q).a_q[k];  e_k likewise
    logits = e_q[:,:,:,None] + e_k[:,:,None,:], symmetrized
    alpha  = softmax(logits, -1); ema update; bias_log = log(clip(ema'))

Exact algebra used (same as the earlier f32 version):
  * sym-logits[i,j] = 0.5*(s_i + s_j) with s = e_q + e_k, so the row
    softmax collapses: alpha[b,k,i,j] = softmax_j(0.5*s[b,k,:])[j],
    independent of i.
  * lrelu(x) = 0.6*x + 0.4*|x| (slope 0.2), so
    0.5*s[b,k,n] = fused[b,n,:] @ wlin[:,k]
                   + 0.2*(a_q[k] . |q_T|) + 0.2*(a_k[k] . |k_T|)
    with wlin[:,k] = 0.3*(W_q[k] @ a_q[k] + W_k[k] @ a_k[k]) host-folded.
  * b_fuse enters only through |q+bq|, |k+bk| (bq = b_fuse@W_q etc) and an
    additive constant in s that the softmax cancels.
  * bias_log content is batch-independent ([K,N,N] broadcast over B).

Sharding: core m owns basis k=m for ALL batches (K-sharded, SPMD, zero
cross-core communication).

Performance structure (memory-regime; ~358 GB/s HBM per core):
  * Outputs are written in bf16 and upcast to f32 on the host during the
    gather step (the kernel's compute is bf16 anyway; measured extra rel
    err ~1.3e-3 alpha / 2.6e-3 bias against a 2e-2 gate). This halves
    write traffic: 8.4 MiB/core instead of 16.8 MiB.
  * Read path: xb0 is its own first DMA so batch-0 compute starts ~3us
    in; b1-b3 follow on the same (sync) queue; b4-b7 + ema ride the
    gpsimd SWDGE queue so the sync queue is free for alpha writes the
    moment batch 0's softmax lands. Weights load on the scalar queue.
  * Queue plan: qSync = xb0, xr1, alpha[0..7], 2 bias quarters;
    qScalar = weights, 2 bias quarters; qGpSimd(SWDGE) = ema, xr2.
    gpsimd engine does only DMA-lib work first, then swaps once to the
    partition-broadcast ucode (lib swaps cost ~1-2us each).
  * Per-batch engine balance: Tensor 7 matmuls (q,k share one 2-bank
    PSUM tile); ACT one |qk| abs + exp+accum; DVE psum->sbuf cast
    (+b_fuse fold — the softmax cancels the resulting constant in s),
    reciprocal, p-normalize, pbar accumulate; gpsimd p-broadcast (bf16).
  * bias tail: ema is held as [128, 4N] (rows 4p+i on partition p), the
    ln(MOM*max(ema + C*pbar, EPS/MOM)) pass runs as 2 chunks of
    [128, 2N] so each DMA descriptor covers 2 consecutive HBM rows
    (2 KB descriptors), 4 DMAs split over the sync+scalar queues.
"""

import sys

import numpy as np

if "/opt/trn_rl_repo" not in sys.path:
    sys.path.insert(0, "/opt/trn_rl_repo")

from contextlib import ExitStack

import ml_dtypes

import concourse.bass as bass
import concourse.tile as tile
from concourse import bacc, mybir
from concourse.bass_utils import run_bass_kernel_spmd

B, N, D, K = 8, 512, 128, 8
R = D
MOM = 0.99
EPS = 1e-6
N_CORES = 8
F32 = mybir.dt.float32
BF16 = mybir.dt.bfloat16
AF = mybir.ActivationFunctionType
ALU = mybir.AluOpType
NPBF = ml_dtypes.bfloat16

WCOLS = 2 * R + 3  # wq | wk | 0.2*aq | 0.2*ak | wlin


def build():
    """Build the SPMD per-core Bass program (identical on all 8 cores)."""
    nc = bacc.Bacc("TRN2", target_bir_lowering=False, debug=False,
                   num_devices=N_CORES)

    # ---- per-core external tensors -------------------------------------
    # xT[d, b*2N + h*N + n]: h=0 desc[b].T, h=1 nve[b].T (bf16, shared)
    xT = nc.dram_tensor("xT", [D, B * 2 * N], BF16, kind="ExternalInput")
    # One combined weight load: wfuse | wpack | b_fuse-column. Separate
    # small weight DMAs (512B descriptors) get starved by packet
    # round-robin against the bulk x reads and complete 5-9us late.
    wall = nc.dram_tensor("wall", [D, 2 * D + WCOLS + 1], BF16,
                          kind="ExternalInput")
    ema = nc.dram_tensor("ema", [N, N], BF16, kind="ExternalInput")  # [m]
    alpha = nc.dram_tensor("alpha", [B, N, N], BF16, kind="ExternalOutput")
    biaso = nc.dram_tensor("bias", [B, N, N], BF16, kind="ExternalOutput")

    NB0 = 1   # batches in the first x DMA
    NB1 = 3   # batches in the second x DMA (sync queue)

    with ExitStack() as ctx:
        tc = ctx.enter_context(tile.TileContext(nc))
        const = ctx.enter_context(tc.tile_pool(name="const", bufs=1))
        work = ctx.enter_context(tc.tile_pool(name="work", bufs=2))
        absp = ctx.enter_context(tc.tile_pool(name="absp", bufs=4))
        psum = ctx.enter_context(tc.tile_pool(name="psum", bufs=1, space="PSUM"))

        wall_sb = const.tile([D, 2 * D + WCOLS + 1], BF16)
        bfuse_sb = const.tile([D, 1], F32)
        cones_sb = const.tile([1, D], F32)
        ema_sb = const.tile([128, 4 * N], BF16)
        x0_sb = const.tile([D, NB0 * 2 * N], BF16)
        x1_sb = const.tile([D, NB1 * 2 * N], BF16)
        x2_sb = const.tile([D, (B - NB0 - NB1) * 2 * N], BF16)
        pbs1 = const.tile([1, N], F32)
        warm_sb = const.tile([1, 16], BF16)
        warm2_sb = const.tile([1, 16], F32)
        warm3_sb = const.tile([8, 16], F32)

        # ---- input DMAs ------------------------------------------------
        # gpsimd: tiny SWDGE warm-up, then the two non-urgent reads, then
        # a partition-broadcast warm-up (one ucode lib swap, early).
        nc.gpsimd.dma_start(warm_sb[:], wall[0:1, 0:16])
        nc.gpsimd.dma_start(
            ema_sb[:].rearrange("p (i n) -> p i n", i=4),
            ema.ap().rearrange("(p i) n -> p i n", p=128))
        nc.gpsimd.dma_start(x2_sb[:], xT[:, (NB0 + NB1) * 2 * N:])
        nc.vector.memset(warm2_sb[:], 0.0)
        nc.gpsimd.partition_broadcast(warm3_sb[:], warm2_sb[:], 8)
        # sync: weights first (they gate all compute and drain in ~0.5us),
        # then batch-0 x (gates the first matmul), then b1-b3. Same-queue
        # FIFO is the priority mechanism.
        nc.sync.dma_start(wall_sb[:], wall[:])
        nc.sync.dma_start(x0_sb[:], xT[:, 0:NB0 * 2 * N])
        nc.sync.dma_start(x1_sb[:], xT[:, NB0 * 2 * N:(NB0 + NB1) * 2 * N])
        # cones: the c-valued row used to replicate c*pbar via the PE
        nc.vector.memset(cones_sb[:], 0.01 / B / MOM)

        wfa_ap = wall_sb[:, 0:D]
        wfb_ap = wall_sb[:, D:2 * D]
        wq_ap = wall_sb[:, 2 * D:2 * D + R]
        wk_ap = wall_sb[:, 2 * D + R:2 * D + 2 * R]
        aq_ap = wall_sb[:, 2 * D + 2 * R:2 * D + 2 * R + 1]
        ak_ap = wall_sb[:, 2 * D + 2 * R + 1:2 * D + 2 * R + 2]
        wlin_ap = wall_sb[:, 2 * D + 2 * R + 2:2 * D + 2 * R + 3]
        # b_fuse column -> f32 once (DVE)
        nc.vector.tensor_copy(bfuse_sb[:], wall_sb[:, 2 * D + WCOLS:])
        bf_ap = bfuse_sb[:, 0:1]

        def x_ap(b):
            if b < NB0:
                return x0_sb[:, b * 2 * N:(b + 1) * 2 * N]
            if b < NB0 + NB1:
                bb = b - NB0
                return x1_sb[:, bb * 2 * N:(bb + 1) * 2 * N]
            bb = b - NB0 - NB1
            return x2_sb[:, bb * 2 * N:(bb + 1) * 2 * N]

        for b in range(B):
            xb = x_ap(b)
            psum_f = psum.tile([D, N], F32, tag="mm", bufs=2)
            nc.tensor.matmul(psum_f[:], wfa_ap, xb[:, 0:N],
                             start=True, stop=False)
            nc.tensor.matmul(psum_f[:], wfb_ap,
                             xb[:, N:2 * N], start=False, stop=True)
            # fused = psum + b_fuse (the wlin.b_fuse constant in s cancels
            # in the softmax; the q/k projections get their bias through
            # fused itself).
            fused_sb = absp.tile([D, N], BF16, tag="fused", bufs=3)
            nc.vector.tensor_scalar_add(fused_sb[:], psum_f[:], bf_ap)
            psum_s = psum.tile([1, N], F32, tag="ps", bufs=2)
            nc.tensor.matmul(psum_s[:], wlin_ap, fused_sb[:],
                             start=True, stop=False)
            # q and k land side by side in one 2-bank PSUM tile so a
            # single ACT Abs covers both.
            psum_qk = psum.tile([D, 2 * N], F32, tag="qk", bufs=2)
            nc.tensor.matmul(psum_qk[:, 0:N], wq_ap, fused_sb[:],
                             start=True, stop=True)
            nc.tensor.matmul(psum_qk[:, N:2 * N], wk_ap, fused_sb[:],
                             start=True, stop=True)
            absqk = absp.tile([D, 2 * N], BF16, tag="abs", bufs=3)
            nc.scalar.activation(absqk[:], psum_qk[:], AF.Abs)
            nc.tensor.matmul(psum_s[:], aq_ap, absqk[:, 0:N],
                             start=False, stop=False)
            nc.tensor.matmul(psum_s[:], ak_ap, absqk[:, N:2 * N],
                             start=False, stop=True)

            # ---- softmax over free dim (no max-shift: |s| is O(3)) -----
            expv = work.tile([1, N], F32, tag="ex", bufs=4)
            sume = work.tile([1, 1], F32, tag="se", bufs=4)
            nc.scalar.activation(expv[:], psum_s[:], AF.Exp,
                                 scale=1.0, accum_out=sume[:])
            rsum = work.tile([1, 1], F32, tag="rs", bufs=4)
            nc.vector.reciprocal(rsum[:], sume[:])

            # p_b (bf16) on one partition, replicated to 128 by gpsimd.
            pnorm = work.tile([1, N], BF16, tag="pn", bufs=4)
            nc.vector.tensor_scalar_mul(pnorm[:], expv[:], rsum[:])
            # pbar partial sum on [1,N] f32 (DVE)
            if b == 0:
                nc.vector.tensor_scalar_mul(pbs1[:], expv[:], rsum[:])
            else:
                nc.vector.scalar_tensor_tensor(
                    pbs1[:], expv[:], rsum[:], pbs1[:],
                    op0=ALU.mult, op1=ALU.add)
            rep_t = absp.tile([128, N], BF16, tag="repsb", bufs=4)
            nc.gpsimd.partition_broadcast(rep_t[:], pnorm[:], 128)
            src = rep_t[:].rearrange(
                "p (o n) -> p o n", o=1).broadcast_to([128, 4, N])
            dst = alpha[b].rearrange("(p i) j -> p i j", p=128)
            # sync engine is idle after its two read issues; a dma_start
            # stalls the issuing engine until the source tile is ready,
            # which is free there.
            nc.sync.dma_start(dst, src)

        # ---- bias_log tail ---------------------------------------------
        # Replicate C*pbar to all 128 partitions on the PE (the gpsimd
        # broadcast would queue behind batch 7's alpha broadcast):
        # psum_pb[p,n] = cones[p] * pbar[n] with cones = C everywhere.
        # bias = ln(MOM * max(ema + C*pbar, EPS/MOM)), 4 pipelined chunks.
        psum_pb = psum.tile([D, N], F32, tag="mm", bufs=2)
        nc.tensor.matmul(psum_pb[:], cones_sb[:], pbs1[:],
                         start=True, stop=True)
        dst_all = biaso.ap().rearrange("b (p i) j -> i p b j", i=4)
        bias_q = [nc.scalar, nc.sync, nc.scalar, nc.sync]
        for i in range(4):
            sl = slice(i * N, (i + 1) * N)
            u = work.tile([128, N], F32, tag="u", bufs=4)
            nc.vector.tensor_tensor(
                u[:], ema_sb[:, sl], psum_pb[:], op=ALU.add)
            v = work.tile([128, N], F32, tag="v", bufs=4)
            nc.vector.tensor_scalar_max(v[:], u[:], EPS / MOM)
            bias_t = work.tile([128, N], BF16, tag="biassb", bufs=4)
            nc.scalar.activation(bias_t[:], v[:], AF.Ln, scale=MOM)
            src = bias_t[:].rearrange(
                "p (o j) -> p o j", o=1).broadcast_to([128, B, N])
            bias_q[i].dma_start(dst_all[i], src)

    nc.compile()
    return nc


_NC_CACHE = None


def _get_nc():
    global _NC_CACHE
    if _NC_CACHE is None:
        _NC_CACHE = build()
    return _NC_CACHE


def make_in_maps(desc_embeddings, name_value_embeddings, W_fuse, b_fuse,
                 W_q, W_k, a, alpha_ema):
    """Host-side sharding / weight prep -> per-core input dicts."""
    desc = np.asarray(desc_embeddings, np.float32)
    nve = np.asarray(name_value_embeddings, np.float32)
    W_fuse = np.asarray(W_fuse, np.float32)
    b_fuse = np.asarray(b_fuse, np.float32)
    W_q = np.asarray(W_q, np.float32)
    W_k = np.asarray(W_k, np.float32)
    a = np.asarray(a, np.float32)
    alpha_ema = np.asarray(alpha_ema, np.float32)

    a_q = a[:, :R, 0]                      # [K,R]
    a_k = a[:, R:, 0]                      # [K,R]
    wlin = 0.3 * (np.einsum("kdr,kr->kd", W_q, a_q)
                  + np.einsum("kdr,kr->kd", W_k, a_k))  # [K,D]

    # xT[d, b*2N + h*N + n]: per-partition-contiguous, batch-major
    xT = np.empty((D, B, 2, N), np.float32)
    xT[:, :, 0, :] = desc.transpose(2, 0, 1)
    xT[:, :, 1, :] = nve.transpose(2, 0, 1)
    xT = np.ascontiguousarray(xT.reshape(D, B * 2 * N)).astype(NPBF)
    # wfuse_t[c, h*D+d] = W_fuse[h*D+c, d]
    wfuse_t = W_fuse.reshape(2, D, D).transpose(1, 0, 2).reshape(D, 2 * D)

    shared = dict(xT=xT)
    in_maps = []
    for m in range(N_CORES):
        wp = np.concatenate(
            [wfuse_t, W_q[m], W_k[m], 0.2 * a_q[m][:, None],
             0.2 * a_k[m][:, None], wlin[m][:, None],
             b_fuse[:, None]], axis=1)
        in_maps.append(dict(
            shared,
            wall=np.ascontiguousarray(wp).astype(NPBF),
            ema=np.ascontiguousarray(alpha_ema[m]).astype(NPBF)))
    return in_maps


def gather(results):
    alpha_full = np.stack(
        [np.asarray(r["alpha"]).astype(np.float32) for r in results], axis=1)
    bias_full = np.stack(
        [np.asarray(r["bias"]).astype(np.float32) for r in results], axis=1)
    return bias_full, alpha_full


def kernel(**inputs):
    nc = _get_nc()
    in_maps = make_in_maps(**inputs)
    res = run_bass_kernel_spmd(nc, in_maps, list(range(N_CORES)))
    return gather(res.results)
